# revision 68
# baseline (speedup 1.0000x reference)
"""AttentionBlockWithSkipConnection Trainium2 kernel.

Full inputs -> full output. Data-parallel over batch B=8 across 8 cores.
Each core computes one batch: GroupNorm -> qkv 1x1conv -> full 4096x4096
attention -> proj 1x1conv -> skip add.

Layout/precision strategy (channel-major middle, fp8 DoubleRow matmuls):
  x^T [C, N] fp8e4      (64 PE transposes of the fp32 x, drains convert;
      x and x^T are each split into FOUR tiles so every consumer's
      DMA/drain wait covers only a quarter of the stream -- single big
      tiles were observed to coarsen Tile's semaphore waits and stall
      whole engine queues)
  GroupNorm stats via bn_stats on the fp8 x^T quarters; group mean/E[x^2]
      via ONE gAB (average+broadcast) matmul; affine folded into the qkv
      weights (fp8): qkv^T = (w*a)^T x^T + bias2. The Sqrt ACT table is
      preloaded at kernel start (the swap would otherwise land mid-chain).
  q,k channel-major fp8 [128, 4, N]; one DoubleRow matmul per (m, qt).
      Emission order: qt0's q -> all k -> qt0's first two logits pairs ->
      remaining q, so the exp stream warms during the prologue tail.
  V token-major directly: v_tm[tok,d] via DoubleRow with lhsT = x^T tile,
      two tiles per PSUM bank with one wide drain; groups 8-15 are
      deferred into qt0's pair loop (rec bank) to fill exp-ramp bubbles.
      The v bias commutes through softmax (weights sum to 1) and is
      folded through Wp into the output bias row bp4.
  logits^T[k,q] = K.T @ Q, one DoubleRow matmul per (qt, kt); two-pair
      PSUM pipeline (4 banks) keeps the PE ~2 pairs ahead of ACT
  expT = exp(logits/16 - 4) -> fp8e4, ONE ACT op per kt pair ([128,2,512]
      spanning 2 banks; the shift keeps exp <= e^3.5 inside fp8 range and
      cancels exactly in the softmax quotient); a few pairs per qt run as
      Schraudolph bit-trick exps on the DVE to relieve ACT.
  denominator = ones[128,2,1] DoubleRow matmul accumulated over kt pairs
      -> colsum [1, 512] on PE; 4 tiny K=1 matmuls transpose it to a
      per-token column, recip on DVE is then a cheap [128,4] op.
  o_un^T = V.T @ expT   (DoubleRow fp8, flash-style PSUM accumulation)
  Normalization happens AFTER the proj (it commutes: per-token scale):
      av_sb = o_un^T cast to bf16 frees the AV banks immediately, so the
      next q tile's AV stream starts with no reciprocal on its path.
  proj TOKEN-major: out[tok,c] via lhsT=av_sb chunks (bf16, FWL); then
      out = proj * recip[token] + (x + bp4) as one fused
      scalar_tensor_tensor per token tile, and DMA out.

PSUM: phase A uses 6 transpose banks + 1; phase B runs under the open
logits pool (4 banks) with 3 qkv banks + 1 small; phase D is exactly 8:
logits pairs 4 + den 1 + rec 1 + av 2, with the proj reusing den/rec at
the tail. DMA queues: x spreads over sync/scalar/gpsimd in consumption
order; weight DMAs avoid the scalar queue entirely because the ACT engine
runs the transpose drains and would stall behind them.
"""

import numpy as np

import concourse.bacc as bacc
import concourse.mybir as mybir
import concourse.tile as tile

N_CORES = 8
B, H, W, C = 8, 64, 64, 256
N = H * W  # 4096 tokens
G = 32  # groups
GS = C // G  # 8 channels per group
EPS = 1e-5
CC = C // 128  # 2 channel chunks
QT = 512  # q tile (free dim of logits/attnv matmuls)
NQ = N // QT  # 8
NK = N // 128  # 32 k tiles
NP = NK // 2  # 16 k-tile pairs (DoubleRow)
F32 = mybir.dt.float32
BF16 = mybir.dt.bfloat16
F8 = mybir.dt.float8e4
DRM = mybir.MatmulPerfMode.DoubleRow
AF = mybir.ActivationFunctionType
EXP_SHIFT = 4.0
I32 = mybir.dt.int32
import math
SCH_A = float(np.float32(2.0 ** 23 / (16.0 * math.log(2.0))))
SCH_B = float(np.float32(127.0 * 2.0 ** 23 - 366393.0
                         - 4.0 * 2.0 ** 23 / math.log(2.0)))
# pairs whose exp runs on the DVE instead of ACT (spread mid-qt, away from
# the qt-boundary tail where the DVE is busy)
DVE_EXP_PAIRS = frozenset({5, 8, 11})


def _build(repeat=1):
    nc = bacc.Bacc(
        "TRN2",
        target_bir_lowering=False,
        debug=False,
        enable_asserts=True,
        num_devices=N_CORES,
    )
    x_d = nc.dram_tensor("x", [N, C], F32, kind="ExternalInput")
    gns_d = nc.dram_tensor("gn_scale", [C], F32, kind="ExternalInput")
    gnb_d = nc.dram_tensor("gn_bias", [C], F32, kind="ExternalInput")
    wq_d = nc.dram_tensor("w_qkv", [C, 3 * C], F32, kind="ExternalInput")
    bq_d = nc.dram_tensor("b_qkv", [3 * C], F32, kind="ExternalInput")
    wp_d = nc.dram_tensor("w_proj", [C, C], F32, kind="ExternalInput")
    bp_d = nc.dram_tensor("b_proj", [C], F32, kind="ExternalInput")
    out_d = nc.dram_tensor("out", [N, C], F32, kind="ExternalOutput")

    # group-aggregation mask: gAB[p, p'] = 1/GS if p, p' share a group --
    # one matmul both averages the 8 channels of a group and broadcasts the
    # result back to all 128 channel rows.
    gAB_np = np.zeros((128, 128), np.float32)
    for p in range(128):
        for p2 in range(p // GS * GS, p // GS * GS + GS):
            gAB_np[p, p2] = 1.0 / GS
    gAB_d = nc.inline_tensor(gAB_np, "gAB")
    ident_d = nc.inline_tensor(np.eye(128, dtype=np.float32), "ident")

    with tile.TileContext(nc) as tc:
        for _ in range(repeat):
            _body(tc, x_d, gns_d, gnb_d, wq_d, bq_d, wp_d, bp_d, out_d,
                  gAB_d, ident_d)
    nc.compile()
    return nc


def _body(tc, x_d, gns_d, gnb_d, wq_d, bq_d, wp_d, bp_d, out_d,
          gAB_d, ident_d):
    nc = tc.nc
    x_tok = x_d.ap().rearrange("(p nt) c -> p nt c", p=128)  # [128, 32, 256]
    out_tok = out_d.ap().rearrange("(p nt) c -> p nt c", p=128)

    with (
        nc.allow_low_precision("mixed-precision attention: bf16/fp8 matmul "
                               "operands, fp32 accumulation throughout"),
        tc.tile_pool(name="consts", bufs=1) as consts,
        tc.tile_pool(name="xtm", bufs=2) as xtm_pool,
        tc.tile_pool(name="xcm", bufs=1) as xcm_pool,
        tc.tile_pool(name="qkvT", bufs=1) as qkvT_pool,
        tc.tile_pool(name="vtm", bufs=1) as vtm_pool,
        tc.tile_pool(name="gn_stats", bufs=2) as gn_stats,
    ):
        # ---- input DMAs: x first (PE transposes gate on it) ----
        ident = consts.tile([128, 128], F32)
        nc.gpsimd.dma_start(out=ident, in_=ident_d.ap())
        # x split into four quarter-tiles: the DMA-completion wait before a
        # transpose covers only its quarter's 4 chunks, so the transpose
        # stream chases the DMA stream closely
        x_tm_q = [
            xtm_pool.tile([128, 4, C], F32, tag=f"x_tm_{i}", name=f"x_tm_{i}")
            for i in range(8)
        ]

        def x_tm(nt0, nt1):
            q = nt0 // 4
            assert nt1 <= (q + 1) * 4
            return x_tm_q[q][:, nt0 - 4 * q : nt1 - 4 * q, :]

        # wp first on the scalar queue: the wp_bf cast's DVE wait resolves
        # against this queue's counter, and a late wp DMA was observed to
        # stall the whole DVE stats stream behind the hoisted cast
        wp_stage = consts.tile([128, CC, C], F32)
        nc.scalar.dma_start(
            out=wp_stage, in_=wp_d.ap().rearrange("(cc p) d -> p cc d", p=128)
        )
        # x over 3 DMA queues, issued in transpose-consumption order so the
        # transpose stream never starves behind a single queue. chunk 1 goes
        # to gpsimd (behind only the tiny ident) instead of scalar (behind
        # the wp DMA), so x tile 0 completes ~0.7us sooner
        dma_engs = [nc.sync, nc.gpsimd, nc.scalar]
        for dchunk in range(16):
            dma_engs[dchunk % 3].dma_start(
                out=x_tm(dchunk * 2, (dchunk + 1) * 2),
                in_=x_tok[:, dchunk * 2 : (dchunk + 1) * 2, :],
            )

        # ---- weights / small constants behind the x chunks. NONE go on the
        # scalar queue: the ACT engine runs the transpose drains, and a DMA
        # ahead of them in its queue would stall the whole transpose phase ----
        gAB = consts.tile([128, 128], F32)
        nc.sync.dma_start(out=gAB, in_=gAB_d.ap())
        wq_stage = consts.tile([128, CC, 3 * C], F32)
        nc.gpsimd.dma_start(
            out=wq_stage, in_=wq_d.ap().rearrange("(cc p) d -> p cc d", p=128)
        )
        wp_bf = consts.tile([128, CC, C], BF16)  # cast emitted in psv loop
        bq = consts.tile([128, 6], F32)
        nc.gpsimd.dma_start(
            out=bq, in_=bq_d.ap().rearrange("(m p) -> p m", p=128)
        )
        bp_stage = consts.tile([1, C], F32)
        nc.sync.dma_start(
            out=bp_stage, in_=bp_d.ap().rearrange("(a c) -> a c", a=1)
        )
        gns = consts.tile([128, CC], F32)
        nc.gpsimd.dma_start(
            out=gns, in_=gns_d.ap().rearrange("(cc p) -> p cc", p=128)
        )
        gnb = consts.tile([128, CC], F32)
        nc.sync.dma_start(
            out=gnb, in_=gnb_d.ap().rearrange("(cc p) -> p cc", p=128)
        )
        ones_raw = consts.tile([128, 128], F32)
        nc.vector.memset(ones_raw, 1.0)
        # denominator DR stationary: [128, 2, 16] so the pair-dim stride is
        # 16 bytes (DoubleRow LDWEIGHTS requires step % 16 == 0); only
        # [:, :, 0:1] is used as the weights column.
        ones8_t = consts.tile([128, 2, 16], F8)
        nc.vector.tensor_copy(out=ones8_t, in_=ones_raw[:, 0:32])
        ones8 = ones8_t[:, :, 0:1]
        ones_col_f = consts.tile([1, 128], F32)
        nc.vector.tensor_copy(out=ones_col_f, in_=ones_raw[0:1, :])
        eps_col = consts.tile([128, 1], F32)
        nc.vector.memset(eps_col, EPS)
        # preload the Sqrt ACT table while the engine is idle waiting for x
        # (else the ~1.3us table swap lands mid stats-chain): build the exp
        # shift as -sqrt(16+eps) ~ -4.0000016; the shift cancels exactly in
        # the softmax quotient so the tiny offset is harmless.
        c16 = consts.tile([128, 1], F32)
        nc.vector.memset(c16, float(EXP_SHIFT * EXP_SHIFT))
        sqrt_warm = consts.tile([128, 1], F32)
        nc.scalar.activation(out=sqrt_warm, in_=c16, func=AF.Sqrt,
                             bias=eps_col)
        nshift_col = consts.tile([128, 1], F32)
        nc.scalar.mul(out=nshift_col, in_=sqrt_warm, mul=-1.0)

        # x_cm split into four tiles (like x_tm): a reader's wait then covers
        # only its quarter's drain writes, so bn_stats/psv/phase-B pipeline
        # against the drain stream instead of waiting for all of it
        x_cm_t = [
            xcm_pool.tile([128, CC, N // 8], F8, tag=f"x_cm_{i}",
                          name=f"x_cm_{i}")
            for i in range(8)
        ]
        XQ = N // 8  # 512 columns per x_cm tile = one bn_stats chunk

        def x_cm3(c0, c1):
            q = c0 // XQ
            assert c1 <= (q + 1) * XQ
            return x_cm_t[q][:, :, c0 - q * XQ : c1 - q * XQ]
        qkvT = qkvT_pool.tile([128, 4, N], F8, tag="qkvT")  # 16KB/partition
        v_tm = vtm_pool.tile([128, NK, C], F8, tag="v_tm")  # 8KB/partition
        wq_f8 = consts.tile([128, CC, 3 * C], F8)  # folded qkv weights
        bp4 = consts.tile([128, 4, C], F32)  # b_proj broadcast 128x4 rows

        with tc.tile_pool(name="pro_ps", bufs=6, space="PSUM") as pro_ps:
            # ---- phase A: transpose x to channel-major fp8; 2 nt (4
            # transposes) batched per full [128,512] PSUM bank so one wide
            # drain amortizes the PSUM-read bubble; bn_stats interleaved ----
            stats = gn_stats.tile([128, CC, 8, 6], F32)
            for g in range(16):
                nt0 = 2 * g
                # bank layout [cc, nt, col] so the drain AP is simple
                ps = pro_ps.tile([128, CC, 2, 128], F32, tag="trx",
                                 name="ps")
                for k in range(2):
                    for cc in range(CC):
                        # x^T via a NORMAL matmul (x tile stationary, ident
                        # moving): pipelines at ~130ns vs ~240ns for the
                        # PE transpose-mode path (SBUF-latency bound, no HAM)
                        nc.tensor.matmul(
                            ps[:, cc, k, :],
                            lhsT=x_tm(nt0 + k, nt0 + k + 1)[
                                :, 0, cc * 128 : (cc + 1) * 128
                            ],
                            rhs=ident,
                            start=True,
                            stop=True,
                        )
                # drains on ACT: the DVE only carries one pair of wide
                # bn_stats per x_cm quarter (below), so ACT's 2x940ns/group
                # fits under the PE's ~2.3us/group transpose rate
                dst = x_cm3(nt0 * 128, (nt0 + 2) * 128)
                nc.scalar.copy(out=dst, in_=ps)
                if nt0 % 4 == 2:
                    # half-quarter complete: 512-sample bn_stats per cc (the
                    # split x_cm quarters keep each stats' wait local to at
                    # most 4 drains, so the DVE stream can't invert)
                    s = nt0 // 4
                    for cc in range(CC):
                        nc.vector.bn_stats(
                            out=stats[:, cc, s, :],
                            in_=x_cm3(s * 512, (s + 1) * 512)[:, cc, :],
                        )

            # ---- groupnorm stats -> per-channel affine (a, b); both cc
            # chunks batched through one set of ops to halve sem latency ----
            ab = gn_stats.tile([128, CC, 2], F32)  # (a, b) per channel
            mv = gn_stats.tile([128, CC, 2], F32, tag="mv")
            for cc in range(CC):
                nc.vector.bn_aggr(out=mv[:, cc, :], in_=stats[:, cc, :, :])
            # in-place: mv[:, :, 1] <- mean^2 + var  (so mv = (mean, E[x^2]))
            msq0 = gn_stats.tile([128, CC, 1], F32, tag="msq0")
            nc.vector.tensor_mul(out=msq0, in0=mv[:, :, 0:1],
                                 in1=mv[:, :, 0:1])
            nc.vector.tensor_add(out=mv[:, :, 1:2], in0=mv[:, :, 1:2],
                                 in1=msq0)
            # one gAB matmul both group-averages and broadcasts back
            chs = pro_ps.tile([128, 4], F32, tag="smm", name="chs", bufs=1)
            nc.tensor.matmul(chs, lhsT=gAB, rhs=mv, start=True, stop=True)
            chs_sb = gn_stats.tile([128, CC, 2], F32, tag="chs_sb")
            nc.vector.tensor_copy(out=chs_sb, in_=chs)
            chs2 = chs_sb
            var = gn_stats.tile([128, CC, 1], F32, tag="var")
            msq = gn_stats.tile([128, CC, 1], F32, tag="msq")
            nc.vector.tensor_mul(out=msq, in0=chs2[:, :, 0:1],
                                 in1=chs2[:, :, 0:1])
            nc.vector.tensor_sub(out=var, in0=chs2[:, :, 1:2], in1=msq)
            nc.scalar.activation(out=var, in_=var, func=AF.Sqrt, bias=eps_col)
            rstd = gn_stats.tile([128, CC, 1], F32, tag="rstd")
            nc.vector.reciprocal(out=rstd, in_=var)
            nc.vector.tensor_mul(
                out=ab[:, :, 0:1], in0=rstd,
                in1=gns.rearrange("p (cc one) -> p cc one", one=1),
            )
            nc.vector.tensor_mul(out=msq, in0=chs2[:, :, 0:1],
                                 in1=ab[:, :, 0:1])
            nc.vector.tensor_sub(
                out=ab[:, :, 1:2], in0=gnb.rearrange("p (cc one) -> p cc one", one=1),
                in1=msq,
            )


        # ---- fold the affine into the qkv weights (fp8):
        # qkv^T = (w*a)^T x^T + (w^T b + b_qkv). V weights first so the
        # v_tm matmuls start ASAP; folds split across ACT and DVE ----
        def fold_m(m, i):
            for cc in range(CC):
                dst = wq_f8[:, cc, m * 128 : (m + 1) * 128]
                src = wq_stage[:, cc, m * 128 : (m + 1) * 128]
                if (2 * i + cc) % 2 == 0:
                    nc.scalar.mul(out=dst, in_=src, mul=ab[:, cc, 0:1])
                else:
                    nc.vector.tensor_scalar_mul(
                        out=dst, in0=src, scalar1=ab[:, cc, 0:1]
                    )

        # the logits pool opens before the second prologue scope: it takes
        # the 4 banks freed by pro_ps, so qt0's first logits pairs can be
        # emitted mid-phase-B and the exp stream starts during the prologue
        with tc.tile_pool(name="lgp", bufs=2, space="PSUM") as lgp:

            def emit_lg2(qt, pair):
                """One [128, 2, 512] logits pair (two DoubleRow matmuls, each
                contracting the full C=256 via the fp8 pair dim) feeding one
                wide ACT exp. Two rotating pair-tiles (4 banks) keep the PE
                ~2 pairs ahead of the exp stream."""
                lg2 = lgp.tile([128, 2, QT], F32, tag="lg2", name="lg2")
                for j in range(2):
                    kt = 2 * pair + j
                    nc.tensor.matmul(
                        lg2[:, j, :],
                        lhsT=qkvT[:, 2:4, kt * 128 : (kt + 1) * 128],
                        rhs=qkvT[:, 0:2, qt * QT : (qt + 1) * QT],
                        start=True,
                        stop=True,
                        perf_mode=DRM,
                    )
                return lg2

            def next_lg2(qt, pair):
                if pair < NP:
                    return emit_lg2(qt, pair)
                if qt + 1 < NQ:
                    return emit_lg2(qt + 1, pair - NP)
                return None

            with (
                tc.tile_pool(name="pro_mm", bufs=4, space="PSUM") as pro_mm,
            ):
                for i, m in enumerate((4, 5)):
                    fold_m(m, i)

                # ---- v token-major directly: one DoubleRow matmul per token
                # tile; v bias folds through the projection into bp4 below.
                # q/k folds and the small bias2 matmuls interleave into the
                # drain-paced psv stream at no extra PE wall time ----
                bias2 = gn_stats.tile([128, 6], F32)

                def emit_bias2(m):
                    psb = pro_mm.tile([128, 2], F32, tag="qkv", name="psb")[:, 0:1]
                    for cc in range(CC):
                        nc.tensor.matmul(
                            psb,
                            lhsT=wq_stage[:, cc, m * 128 : (m + 1) * 128],
                            rhs=ab[:, cc, 1:2],
                            start=(cc == 0),
                            stop=(cc == CC - 1),
                        )
                    nc.vector.tensor_add(
                        out=bias2[:, m : m + 1], in0=psb, in1=bq[:, m : m + 1]
                    )

                def emit_psv(g, eng, pool=None, tag="qkv"):
                    """Two v token tiles per full [128, 2, 256] PSUM bank
                    with one wide drain on `eng`."""
                    psv = (pool or pro_mm).tile([128, 2, C], F32, tag=tag,
                                                name="psv")
                    for k in range(2):
                        nt = 2 * g + k
                        nc.tensor.matmul(
                            psv[:, k, :],
                            lhsT=x_cm3(nt * 128, (nt + 1) * 128),
                            rhs=wq_f8[:, 0:CC, 2 * C : 3 * C],
                            start=True,
                            stop=True,
                            perf_mode=DRM,
                        )
                    eng_copy = (nc.vector.tensor_copy if eng == "v"
                                else nc.scalar.copy)
                    eng_copy(out=v_tm[:, 2 * g : 2 * g + 2, :], in_=psv)

                for g in range(8):
                    emit_psv(g, "v" if g % 2 == 0 else "s")
                    if g == 0:
                        # wp cast here, NOT in the consts block: emitted any
                        # earlier, the scheduler slots it into the bn_stats
                        # stream where its wp-DMA wait stalls the DVE queue
                        nc.vector.tensor_copy(out=wp_bf, in_=wp_stage)
                    if g == 1:
                        for i, m in enumerate((0, 1)):
                            fold_m(m, i)
                    if g == 2:
                        for i, m in enumerate((2, 3)):
                            fold_m(m, i)
                    if 2 <= g:
                        emit_bias2(g - 2)  # m = 0..5 across g = 2..7

                # ---- phase B: q,k channel-major (+ bias2), one DoubleRow
                # fp8 matmul per (m, qt). Emission order: qt0's q, then all
                # k, then qt0's first two logits pairs (so the exp stream
                # starts while the rest of q is still streaming), then q
                # for qt 1..7 ----
                def emit_phase_b(m, qt, eng=None):
                    ps = pro_mm.tile([128, QT], F32, tag="qkv", name="ps")
                    nc.tensor.matmul(
                        ps,
                        lhsT=wq_f8[:, 0:CC, m * 128 : (m + 1) * 128],
                        rhs=x_cm3(qt * QT, (qt + 1) * QT),
                        start=True,
                        stop=True,
                        perf_mode=DRM,
                    )
                    if eng is None:
                        eng = "s" if qt % 2 == 0 else "v"
                    if eng == "s":
                        nc.scalar.activation(
                            out=qkvT[:, m, qt * QT : (qt + 1) * QT],
                            in_=ps,
                            func=AF.Identity,
                            bias=bias2[:, m : m + 1],
                        )
                    else:
                        nc.vector.tensor_scalar_add(
                            out=qkvT[:, m, qt * QT : (qt + 1) * QT],
                            in0=ps,
                            scalar1=bias2[:, m : m + 1],
                        )

                emit_phase_b(0, 0)
                emit_phase_b(1, 0)
                for m in (2, 3):
                    for qt in range(NQ):
                        emit_phase_b(m, qt)

                # effective output bias row: bp + Wp^T @ bias_v  (v bias
                # commutes through softmax: attention weights sum to 1)
                bpv_ps = pro_mm.tile([128, C], F32, tag="qkv",
                                     name="bpv_ps")[0:1, :]
                for cc in range(CC):
                    nc.tensor.matmul(
                        bpv_ps,
                        lhsT=bias2[:, 4 + cc : 5 + cc],
                        rhs=wp_stage[:, cc, :],
                        start=(cc == 0),
                        stop=(cc == CC - 1),
                    )
                bpe_row = gn_stats.tile([1, C], F32, tag="bpe_row")
                nc.vector.tensor_add(out=bpe_row, in0=bpv_ps, in1=bp_stage)
                bp_ps = pro_mm.tile([128, C], F32, tag="qkv",
                                    name="bp_ps")
                nc.tensor.matmul(
                    bp_ps, lhsT=ones_col_f, rhs=bpe_row,
                    start=True, stop=True,
                )
                for r in range(4):
                    if r % 2 == 0:
                        nc.vector.tensor_copy(out=bp4[:, r, :], in_=bp_ps)
                    else:
                        nc.scalar.copy(out=bp4[:, r, :], in_=bp_ps)

                lg2_cur = emit_lg2(0, 0)
                lg2_nxt = emit_lg2(0, 1)
                # q-rest adds split across both engines so the pro_mm banks
                # recycle at two-engine pace; v groups 8-15 move into qt0's
                # pair loop (rec bank) where they fill the exp-ramp bubbles
                for m in (0, 1):
                    for qt in range(1, NQ):
                        emit_phase_b(m, qt)

            # ---- phase D: attention + proj + skip, per q tile ----
            with (
                tc.tile_pool(name="pmisc", bufs=1, space="PSUM") as pmisc,
                tc.tile_pool(name="avp", bufs=1, space="PSUM") as avp,
                tc.tile_pool(name="expp", bufs=3) as expp,
                tc.tile_pool(name="owork", bufs=2) as owork,
            ):
                for qt in range(NQ):
                    av_ps = [
                        avp.tile([128, QT], F32, tag=f"av{cc}",
                                 name=f"av{cc}")
                        for cc in range(CC)
                    ]
                    # skip + bias precombined early, off the tail critical
                    # path
                    xb4 = owork.tile([128, 4, C], F32, tag="xb4")
                    nc.vector.tensor_add(
                        out=xb4, in0=bp4, in1=x_tm(qt * 4, (qt + 1) * 4)
                    )
                    # denominator bank: matmul writes row 0; the full
                    # [128, 512] shape lets the proj reuse the bank (tag den)
                    den_t = pmisc.tile([128, QT], F32, tag="den",
                                       name="den_t")
                    den = den_t[0:1, :]
                    # qt0 ramps the exp pipeline from scratch: give the DVE
                    # two extra early pairs so ACT+DVE fill it in parallel
                    dve_pairs = DVE_EXP_PAIRS | ({0, 2} if qt == 0 else set())

                    for pair in range(NP):
                        if qt == 0 and pair < 8:
                            # v groups 8-15, deferred from the prologue into
                            # the rec bank: AV for pair p only needs group p,
                            # so group 8+pair lands 8 pairs ahead of its use
                            emit_psv(8 + pair, "v" if pair % 2 else "s",
                                     pool=pmisc, tag="rec")
                        expT2 = expp.tile([128, 2, QT], F8, tag="expT2",
                                          name="expT2")
                        if pair in dve_pairs:
                            # Schraudolph fast exp on the DVE (ACT relief):
                            # i32(x*Af+Bf) bits ~ exp(x/16-4), +-3% rel
                            ti = expp.tile([128, 2, QT], I32, tag="ti",
                                           name="ti")
                            nc.vector.tensor_scalar(
                                out=ti, in0=lg2_cur,
                                scalar1=SCH_A, scalar2=SCH_B,
                                op0=mybir.AluOpType.mult,
                                op1=mybir.AluOpType.add)
                            nc.vector.tensor_copy(out=expT2,
                                                  in_=ti.bitcast(F32))
                        else:
                            # one wide exp over both banks; the -4 shift
                            # keeps the fp8 numerator in range, cancels in
                            # the softmax quotient.
                            nc.scalar.activation(
                                out=expT2,
                                in_=lg2_cur,
                                func=AF.Exp,
                                scale=1.0 / 16.0,
                                bias=nshift_col,
                            )
                        lg2_cur = lg2_nxt
                        lg2_nxt = next_lg2(qt, pair + 2)
                        for cc in range(CC):
                            nc.tensor.matmul(
                                av_ps[cc],
                                lhsT=v_tm[:, 2 * pair : 2 * pair + 2,
                                          cc * 128 : (cc + 1) * 128],
                                rhs=expT2,
                                start=(pair == 0),
                                stop=(pair == NP - 1),
                                perf_mode=DRM,
                            )
                        nc.tensor.matmul(
                            den,
                            lhsT=ones8,
                            rhs=expT2,
                            start=(pair == 0),
                            stop=(pair == NP - 1),
                            perf_mode=DRM,
                        )

                    # ---- tail: per-token recip column; normalize AFTER the
                    # proj so av_ps banks free as soon as they're cast (no
                    # recip on the PE/AV critical path) ----
                    den_sb = owork.tile([1, QT], F32, tag="den_sb")
                    nc.vector.tensor_copy(out=den_sb, in_=den)
                    # den row -> per-token column via 4 tiny K=1 matmuls
                    # (into the rec bank: the den bank frees for pjB as soon
                    # as den_sb is read)
                    recT = pmisc.tile([128, 4], F32, tag="rec", name="recT")
                    for t4 in range(4):
                        nc.tensor.matmul(
                            recT[:, t4 : t4 + 1],
                            lhsT=den_sb[0:1, t4 * 128 : (t4 + 1) * 128],
                            rhs=ones_col_f[0:1, 0:1],
                            start=True,
                            stop=True,
                        )
                    recip4 = owork.tile([128, 4], F32, tag="recip4")
                    nc.vector.reciprocal(out=recip4, in_=recT)

                    # unnormalized o in bf16 for the proj matmuls
                    av_sb = owork.tile([128, CC, QT], BF16, tag="av_sb")
                    nc.vector.tensor_copy(out=av_sb[:, 0, :], in_=av_ps[0])
                    nc.scalar.copy(out=av_sb[:, 1, :], in_=av_ps[1])

                    # proj token-major; pjB first (den bank, free earliest),
                    # pjA into the rec bank after recip4 consumes recT
                    pjB = pmisc.tile([128, QT], F32, tag="den", name="pjB")
                    pjA = pmisc.tile([128, QT], F32, tag="rec", name="pjA")
                    for tt in (2, 3, 0, 1):
                        bank = pjA if tt < 2 else pjB
                        seg = bank[:, (tt % 2) * C : (tt % 2 + 1) * C]
                        for cc in range(CC):
                            nc.tensor.matmul(
                                seg,
                                lhsT=av_sb[:, cc, tt * 128 : (tt + 1) * 128],
                                rhs=wp_bf[:, cc, :],
                                start=(cc == 0),
                                stop=(cc == CC - 1),
                            )

                    # out = proj * recip[token] + (x + bp), fused per token
                    # tile; pjB's halves (done first) flush in their own DMA
                    # so the final store overlaps the pjA-side ops
                    out_sb = owork.tile([128, 4, C], F32, tag="out_sb")
                    for tt in (2, 3, 0, 1):
                        bank = pjA if tt < 2 else pjB
                        seg = bank[:, (tt % 2) * C : (tt % 2 + 1) * C]
                        nc.vector.scalar_tensor_tensor(
                            out=out_sb[:, tt, :],
                            in0=seg,
                            scalar=recip4[:, tt : tt + 1],
                            in1=xb4[:, tt, :],
                            op0=mybir.AluOpType.mult,
                            op1=mybir.AluOpType.add,
                        )
                        if tt == 3:
                            nc.sync.dma_start(
                                out=out_tok[:, qt * 4 + 2 : qt * 4 + 4, :],
                                in_=out_sb[:, 2:4, :],
                            )
                    nc.sync.dma_start(
                        out=out_tok[:, qt * 4 : qt * 4 + 2, :],
                        in_=out_sb[:, 0:2, :],
                    )


_NC = None


def _get_nc():
    global _NC
    if _NC is None:
        _NC = _build()
    return _NC


_RUNNER = None
_ZEROS_FN = None

IN_NAMES = ["x", "gn_scale", "gn_bias", "w_qkv", "b_qkv", "w_proj", "b_proj"]


def _get_runner():
    """Cached jitted shard_map executable over the 8 cores (the equivalent of
    run_bass_kernel_spmd's axon path, but built once instead of per call)."""
    global _RUNNER
    if _RUNNER is not None:
        return _RUNNER
    import jax
    from jax.sharding import Mesh, PartitionSpec
    from jax.experimental.shard_map import shard_map
    from concourse import bass2jax

    nc = _get_nc()
    bass2jax.install_neuronx_cc_hook()

    in_names = list(IN_NAMES) + ["out"]
    if nc.partition_id_tensor is not None:
        in_names.append(nc.partition_id_tensor.name)

    def _body_fn(*args):
        operands = list(args)
        if nc.partition_id_tensor is not None:
            operands.append(bass2jax.partition_id_tensor())
        outs = bass2jax._bass_exec_p.bind(
            *operands,
            out_avals=(jax.core.ShapedArray((N, C), np.float32),),
            in_names=tuple(in_names),
            out_names=("out",),
            lowering_input_output_aliases=(),
            sim_require_finite=True,
            sim_require_nnan=True,
            nc=nc,
        )
        return tuple(outs)

    devices = jax.devices()[:N_CORES]
    mesh = Mesh(np.asarray(devices), ("core",))
    in_specs = (PartitionSpec("core"),) * (len(IN_NAMES) + 1)
    out_specs = (PartitionSpec("core"),)
    sharded = jax.jit(
        shard_map(
            _body_fn, mesh=mesh, in_specs=in_specs, out_specs=out_specs,
            check_rep=False,
        ),
        donate_argnums=(len(IN_NAMES),),
        keep_unused=True,
    )
    _RUNNER = sharded
    return _RUNNER


def kernel(x, gn_scale, gn_bias, w_qkv, b_qkv, w_proj, b_proj):
    sharded = _get_runner()
    x = np.ascontiguousarray(np.asarray(x, dtype=np.float32).reshape(B * N, C))
    shared = {
        "gn_scale": np.asarray(gn_scale, np.float32),
        "gn_bias": np.asarray(gn_bias, np.float32),
        "w_qkv": np.ascontiguousarray(np.asarray(w_qkv, np.float32)),
        "b_qkv": np.asarray(b_qkv, np.float32),
        "w_proj": np.ascontiguousarray(np.asarray(w_proj, np.float32)),
        "b_proj": np.asarray(b_proj, np.float32),
    }
    # shard_map slices axis 0 across cores: x gets its own batch; the shared
    # weights are tiled 8x so every core sees an identical copy.
    concat = [x]
    for name in IN_NAMES[1:]:
        a = shared[name]
        concat.append(np.concatenate([a] * N_CORES, axis=0))
    # donated output buffer, created on-device (saves a 32MB host->device
    # transfer through the axon tunnel every call)
    import jax
    import jax.numpy as jnp
    from jax.sharding import Mesh, NamedSharding, PartitionSpec

    global _ZEROS_FN
    if _ZEROS_FN is None:
        mesh = Mesh(np.asarray(jax.devices()[:N_CORES]), ("core",))
        sh = NamedSharding(mesh, PartitionSpec("core"))
        _ZEROS_FN = jax.jit(
            lambda: jnp.zeros((N_CORES * N, C), jnp.float32), out_shardings=sh
        )
    zeros = _ZEROS_FN()
    (out,) = sharded(*concat, zeros)
    return np.asarray(out).reshape(B, H, W, C)

